# revision 1
# baseline (speedup 1.0000x reference)
"""Multi-head causal attention on 8 Trainium2 NeuronCores.

Sharding: core c handles batch b = c // 2 and head-group g = c % 2
(8 of 16 heads, i.e. 512 of 1024 projection columns).  QKV projections,
attention and the output projection partial run per-core; the two cores
of a batch pair-ReduceScatter their partial outputs.

Everything on-device is computed in a transposed layout (seq on the
free dim) so no PE transposes are needed anywhere:
  xT [D, L] (host-pre-transposed) -> qT/kT [512, L] -> S^T [keys, q]
  -> P^T = exp(S^T) (bf16) -> attn^T = (v|ones)^T @ P^T (Z row free)
  -> out^T = Wo^T @ attn_norm^T.  Host un-transposes the result.
"""

import sys, types

sys.path.insert(0, "/opt/trn_rl_repo")

# antenv.axon_hooks is missing in this image; inject it so trace=True can
# reach the NTFF profiling hook (used by test.py, off by default).
if "antenv.axon_hooks" not in sys.modules:
    _hook_mod = types.ModuleType("antenv.axon_hooks")
    _hook_mod._hook = None
    def _set_hook(h):
        _hook_mod._hook = h
    def _get_hook():
        return _hook_mod._hook
    _hook_mod.set_axon_ntff_profile_hook = _set_hook
    _hook_mod.get_axon_ntff_profile_hook = _get_hook
    sys.modules["antenv.axon_hooks"] = _hook_mod
    try:
        import antenv
        antenv.axon_hooks = _hook_mod
        from trn_agent_boot.trn_boot import _ntff_profile_via_ctypes
        _set_hook(_ntff_profile_via_ctypes("/opt/axon/libaxon_pjrt.so"))
    except Exception:
        pass

import numpy as np
import ml_dtypes
import concourse.bass as bass
import concourse.mybir as mybir
import concourse.tile as tile
from concourse import bacc
from concourse.bass_utils import run_bass_kernel_spmd

B, L, D, H = 4, 2048, 1024, 16
DH = 64
N_CORES = 8
NH = 8          # heads per core
HC = NH * DH    # 512 projection cols per core
QC = 512        # q-chunk
KT = 128        # k-tile
P = 128

F32 = mybir.dt.float32
F32R = mybir.dt.float32r
BF16 = mybir.dt.bfloat16

TRACE = False
LAST_EXEC_NS = None
_NC = None


def build_nc(seq_len=L):
    Ls = seq_len
    NQC = Ls // QC
    NKT = Ls // KT
    NDS = D // P       # 8 contraction tiles for projections
    nc = bacc.Bacc()

    xT = nc.declare_dram_parameter("xT", [D, Ls], F32R, isOutput=False)
    wq = nc.declare_dram_parameter("wq", [D, HC], F32R, isOutput=False)
    wk = nc.declare_dram_parameter("wk", [D, HC], F32R, isOutput=False)
    wv = nc.declare_dram_parameter("wv", [D, HC], F32R, isOutput=False)
    wo = nc.declare_dram_parameter("wo", [HC, D], BF16, isOutput=False)
    bq = nc.declare_dram_parameter("bq", [P, HC // P], F32, isOutput=False)
    bk = nc.declare_dram_parameter("bk", [P, HC // P], F32, isOutput=False)
    bv = nc.declare_dram_parameter("bv", [P, HC], F32, isOutput=False)
    bo = nc.declare_dram_parameter("bo", [P, D // P], F32, isOutput=False)
    m01 = nc.declare_dram_parameter("m01", [P, 4 * QC], BF16, isOutput=False)
    outTh = nc.declare_dram_parameter("outTh", [D // 2, Ls], F32, isOutput=True)

    partT = nc.dram_tensor("partT", [D, Ls], F32)
    rs_out = nc.dram_tensor("rs_out", [D // 2, Ls], F32)

    scale = 1.0 / np.sqrt(np.float32(DH))

    from contextlib import ExitStack
    with nc.allow_low_precision(reason="f32r matmul inputs; bf16 P/V by design"), \
         tile.TileContext(nc) as tc, ExitStack() as ctx:
        consts = ctx.enter_context(tc.tile_pool(name="consts", bufs=1))
        wpool = ctx.enter_context(tc.tile_pool(name="wpool", bufs=1))
        kvres = ctx.enter_context(tc.tile_pool(name="kvres", bufs=1))
        xtp = ctx.enter_context(tc.tile_pool(name="xtp", bufs=8))
        qtp = ctx.enter_context(tc.tile_pool(name="qtp", bufs=8))
        ptp = ctx.enter_context(tc.tile_pool(name="ptp", bufs=8))
        anp = ctx.enter_context(tc.tile_pool(name="anp", bufs=8))
        otp = ctx.enter_context(tc.tile_pool(name="otp", bufs=3))
        zrp = ctx.enter_context(tc.tile_pool(name="zrp", bufs=2))
        bzsb = ctx.enter_context(tc.tile_pool(name="bzsb", bufs=2))
        anodd = ctx.enter_context(tc.tile_pool(name="anodd", bufs=2))
        zdp = ctx.enter_context(tc.tile_pool(name="zdp", bufs=4, space="DRAM"))
        scratch = ctx.enter_context(tc.tile_pool(name="scratch", bufs=2, space="PSUM"))
        stp = ctx.enter_context(tc.tile_pool(name="stp", bufs=2, space="PSUM"))
        accp = ctx.enter_context(tc.tile_pool(name="accp", bufs=2, space="PSUM"))

        if True:
            # ---- constants ----
            bq_sb = consts.tile([P, HC // P], F32, tag="bq")
            bk_sb = consts.tile([P, HC // P], F32, tag="bk")
            bv_sb = consts.tile([P, HC], F32, tag="bv")
            bo_sb = consts.tile([P, D // P], F32, tag="bo")
            m01_sb = consts.tile([P, 4, QC], BF16, tag="m01")
            ones_sb = consts.tile([P, DH], F32, tag="ones")
            nc.sync.dma_start(out=bq_sb, in_=bq[:, :])
            nc.sync.dma_start(out=bk_sb, in_=bk[:, :])
            nc.sync.dma_start(out=bv_sb, in_=bv[:, :])
            nc.sync.dma_start(out=bo_sb, in_=bo[:, :])
            nc.sync.dma_start(out=m01_sb, in_=m01[:, :].rearrange("p (m q) -> p m q", m=4))
            nc.vector.memset(ones_sb, 1.0)

            # ---- weights resident ----
            wq_sb = [wpool.tile([P, HC], F32R, tag=f"wq{ds}", name=f"wq{ds}") for ds in range(NDS)]
            wk_sb = [wpool.tile([P, HC], F32R, tag=f"wk{ds}", name=f"wk{ds}") for ds in range(NDS)]
            wv_sb = [wpool.tile([P, HC], F32R, tag=f"wv{ds}", name=f"wv{ds}") for ds in range(NDS)]
            wo_sb = [wpool.tile([P, D], BF16, tag=f"wo{t}", name=f"wo{t}") for t in range(HC // P)]
            for ds in range(NDS):
                nc.sync.dma_start(out=wq_sb[ds], in_=wq[ds * P:(ds + 1) * P, :])
                nc.sync.dma_start(out=wk_sb[ds], in_=wk[ds * P:(ds + 1) * P, :])
                nc.sync.dma_start(out=wv_sb[ds], in_=wv[ds * P:(ds + 1) * P, :])
            for t in range(HC // P):
                nc.sync.dma_start(out=wo_sb[t], in_=wo[t * P:(t + 1) * P, :])

            # ---- resident kT and v ----
            kT_sb = [kvres.tile([P, Ls], F32R, tag=f"kT{t}", name=f"kT{t}") for t in range(HC // P)]
            # v: per key-tile [128, NH, 65] bf16; cols 0..63 = v, col 64 = ones
            # (the ones column makes the AV matmul emit softmax Z in row 64)
            v_sb = [kvres.tile([P, NH, 65], BF16, tag=f"v{kt}", name=f"v{kt}") for kt in range(NKT)]
            for kt in range(NKT):
                nc.vector.memset(v_sb[kt], 1.0)

            # ---- per-chunk: projections for chunk s, then attention and
            # output projection for q-chunk c=s (causal => only needs k/v
            # from chunks <= s) ----
            xT_t = {}
            qT_t = {}
            attn_by_chunk = {}
            for s in range(NQC):
                for ds in range(NDS):
                    xt = xtp.tile([P, QC], F32R, tag="xT")
                    nc.sync.dma_start(
                        out=xt, in_=xT[ds * P:(ds + 1) * P, s * QC:(s + 1) * QC])
                    xT_t[(ds, s)] = xt

                for t in range(HC // P):
                    # qT tile [128 outcol, QC seq]
                    pq = scratch.tile([P, QC], F32, tag="pacc")
                    for ds in range(NDS):
                        nc.tensor.matmul(
                            pq,
                            wq_sb[ds][:, t * P:(t + 1) * P],
                            xT_t[(ds, s)],
                            start=(ds == 0), stop=(ds == NDS - 1))
                    qt = qtp.tile([P, QC], F32R, tag="qT")
                    # scale q by 1/sqrt(dh) here; add bias then scale:
                    # (q+b)*s = func(in*s + b*s) with pre-scaled bias
                    nc.scalar.activation(
                        out=qt, in_=pq,
                        func=mybir.ActivationFunctionType.Identity,
                        bias=bq_sb[:, t:t + 1], scale=1.0)
                    qT_t[(t, s)] = qt

                    pk = scratch.tile([P, QC], F32, tag="pacc")
                    for ds in range(NDS):
                        nc.tensor.matmul(
                            pk,
                            wk_sb[ds][:, t * P:(t + 1) * P],
                            xT_t[(ds, s)],
                            start=(ds == 0), stop=(ds == NDS - 1))
                    nc.scalar.activation(
                        out=kT_sb[t][:, s * QC:(s + 1) * QC], in_=pk,
                        func=mybir.ActivationFunctionType.Identity,
                        bias=bk_sb[:, t:t + 1], scale=1.0)

                # v for the 4 key-tiles of this seq chunk
                for sub in range(QC // P):
                    kt = s * (QC // P) + sub
                    pv = scratch.tile([P, HC], F32, tag="pacc")
                    for ds in range(NDS):
                        nc.tensor.matmul(
                            pv,
                            xT_t[(ds, s)][:, sub * P:(sub + 1) * P],
                            wv_sb[ds],
                            start=(ds == 0), stop=(ds == NDS - 1))
                    nc.vector.tensor_add(
                        v_sb[kt][:, :, 0:64],
                        pv[:].rearrange("p (h d) -> p h d", h=NH),
                        bv_sb[:].rearrange("p (h d) -> p h d", h=NH))

                # ---- attention + output projection for q-chunk c = s ----
                c = s
                njt = min(4 * c + 4, NKT)     # causal: k-tiles 0..4c+3
                ngrp = (njt + 1) // 2
                attn_n = {}
                for t in range(HC // P):
                    an_t = anp.tile([P, QC], BF16, tag="an")
                    for par in range(2):
                        h = 2 * t + par
                        # S^T and P^T for all k-tile groups of this head
                        pts = []
                        for g in range(ngrp):
                            st = stp.tile([P, 2 * QC], F32, tag="st")
                            for half in range(2):
                                j = 2 * g + half
                                if j >= njt:
                                    continue
                                nc.tensor.matmul(
                                    st[:, half * QC:(half + 1) * QC],
                                    kT_sb[t][par * DH:(par + 1) * DH,
                                             j * KT:(j + 1) * KT],
                                    qT_t[(t, c)][par * DH:(par + 1) * DH, :],
                                    start=True, stop=True)
                            pt = ptp.tile([P, 2 * QC], BF16, tag="pt")
                            # exp(scale * s)
                            ncols = QC * (2 if 2 * g + 1 < njt else 1)
                            nc.scalar.activation(
                                out=pt[:, :ncols], in_=st[:, :ncols],
                                func=mybir.ActivationFunctionType.Exp,
                                scale=float(scale))
                            pts.append(pt)
                            for half in range(2):
                                j = 2 * g + half
                                if j >= njt or j < 4 * c:
                                    continue
                                m = j - 4 * c
                                nc.vector.tensor_mul(
                                    pt[:, half * QC:(half + 1) * QC],
                                    pt[:, half * QC:(half + 1) * QC],
                                    m01_sb[:, m, :])
                        # AV with fused ones column -> rows 0..63 attn, row 64 = Z
                        acc = accp.tile([P, QC], F32, tag="acc")
                        for j in range(njt):
                            nc.tensor.matmul(
                                acc[0:65, :],
                                v_sb[j][:, h, :],
                                pts[j // 2][:, (j % 2) * QC:(j % 2 + 1) * QC],
                                start=(j == 0), stop=(j == njt - 1))
                        # normalization: zrec = 1/Z, broadcast over 64 rows via PE
                        zrec = zrp.tile([P, QC], F32, tag="zrec")
                        nc.vector.reciprocal(
                            out=zrec[64:65, :], in_=acc[64:65, :])
                        bzs = bzsb.tile([P, QC], F32, tag="bzs")
                        zrow = zdp.tile([1, QC], F32, tag="zd", name="zrow")
                        nc.sync.dma_start(out=zrow, in_=zrec[64:65, :])
                        nc.sync.dma_start(out=bzs[0:DH, :].unsqueeze(1),
                                          in_=zrow.partition_broadcast(DH))
                        if par == 0:
                            nc.vector.tensor_mul(
                                an_t[0:DH, :], acc[0:DH, :], bzs[0:DH, :])
                        else:
                            an_o = anodd.tile([DH, QC], BF16, tag="anodd")
                            nc.vector.tensor_mul(
                                an_o, acc[0:DH, :], bzs[0:DH, :])
                            # shift to partitions 64..127 (DMA can cross lanes)
                            nc.sync.dma_start(out=an_t[DH:P, :], in_=an_o)
                    attn_n[t] = an_t

                attn_by_chunk[c] = attn_n
                # output projection, delayed one chunk so the (slow) softmax
                # normalization chain of chunk c overlaps proj matmuls of c+1
                for oc in ([c - 1] if c + 1 < NQC else [c - 1, c]):
                    if oc < 0:
                        continue
                    an_c = attn_by_chunk.pop(oc)
                    for o in range(D // P):
                        po = scratch.tile([P, QC], F32, tag="pacc")
                        for t in range(HC // P):
                            nc.tensor.matmul(
                                po,
                                wo_sb[t][:, o * P:(o + 1) * P],
                                an_c[t],
                                start=(t == 0), stop=(t == HC // P - 1))
                        ot = otp.tile([P, QC], F32, tag="ot")
                        nc.scalar.activation(
                            out=ot, in_=po,
                            func=mybir.ActivationFunctionType.Identity,
                            bias=bo_sb[:, o:o + 1], scale=1.0)
                        nc.sync.dma_start(
                            out=partT[o * P:(o + 1) * P, oc * QC:(oc + 1) * QC], in_=ot)

    with nc.Block() as block, nc.semaphore("cc_sem") as cc_sem, \
         nc.semaphore("dma_sem") as dma_sem:
        @block.gpsimd
        def _(gpsimd):
            gpsimd.collective_compute(
                "ReduceScatter", mybir.AluOpType.add,
                replica_groups=[[0, 1], [2, 3], [4, 5], [6, 7]],
                ins=[partT[:, :]], outs=[rs_out[:, :]],
            ).then_inc(cc_sem, 1)
            gpsimd.wait_ge(cc_sem, 1)
            gpsimd.dma_start(out=outTh[:, :], in_=rs_out[:, :]).then_inc(dma_sem, 16)
            gpsimd.wait_ge(dma_sem, 16)

    nc.compile()
    return nc


def _make_in_maps(x, Wq, bq, Wk, bk, Wv, bv, Wo, bo, mask):
    ref = np.tril(np.ones((L, L), dtype=np.int32))[None, None]
    assert np.array_equal(np.asarray(mask), ref), "mask must be causal"

    # m01 patterns for the 4 diagonal k-tiles of a q-chunk:
    # pattern_m[p, f] = 1 if p <= f - 128*m
    pf = np.arange(QC)[None, :] - np.arange(P)[:, None]
    m01 = np.concatenate(
        [(pf >= 128 * m).astype(np.float32) for m in range(4)], axis=1)

    in_maps = []
    for c in range(N_CORES):
        b, g = c // 2, c % 2
        cols = slice(HC * g, HC * g + HC)
        in_maps.append({
            "xT": np.ascontiguousarray(np.asarray(x[b]).T),
            "wq": np.ascontiguousarray(np.asarray(Wq)[:, cols]),
            "wk": np.ascontiguousarray(np.asarray(Wk)[:, cols]),
            "wv": np.ascontiguousarray(np.asarray(Wv)[:, cols]),
            "wo": np.ascontiguousarray(np.asarray(Wo)[cols, :]).astype(ml_dtypes.bfloat16),
            "bq": np.ascontiguousarray(np.asarray(bq)[cols].reshape(HC // P, P).T),
            "bk": np.ascontiguousarray(np.asarray(bk)[cols].reshape(HC // P, P).T),
            "bv": np.ascontiguousarray(
                np.broadcast_to(np.asarray(bv)[cols], (P, HC))),
            "bo": np.ascontiguousarray(
                (np.asarray(bo) / 2.0).reshape(D // P, P).T.astype(np.float32)),
            "m01": m01.astype(ml_dtypes.bfloat16),
        })
    return in_maps


def kernel(x, Wq, bq, Wk, bk, Wv, bv, Wo, bo, mask):
    global _NC, LAST_EXEC_NS
    if _NC is None:
        _NC = build_nc()
    in_maps = _make_in_maps(x, Wq, bq, Wk, bk, Wv, bv, Wo, bo, mask)
    r = run_bass_kernel_spmd(
        _NC, in_maps, core_ids=list(range(N_CORES)), trace=TRACE)
    LAST_EXEC_NS = r.exec_time_ns
    out = np.empty((B, L, D), dtype=np.float32)
    for b in range(B):
        outT = np.concatenate(
            [r.results[2 * b]["outTh"], r.results[2 * b + 1]["outTh"]], axis=0)
        out[b] = outT.T
    return out



# revision 5
# speedup vs baseline: 1.4323x; 1.4323x over previous
"""Multi-head causal attention on 8 Trainium2 NeuronCores.

Sharding: core c handles batch b = c // 2 and head-group g = c % 2
(8 of 16 heads, i.e. 512 of 1024 projection columns).  QKV projections
and attention run per-core in bf16; the two cores of a batch pair
exchange their attention outputs with a per-chunk AllGather (bf16,
overlapped with the next chunk's compute) and each core then computes
the output projection over the full 1024 attention features for its
half of the output columns — no end-of-kernel reduce, no final copy.

Everything on-device is computed in a transposed layout (seq on the
free dim) so no PE transposes are needed anywhere:
  xT [D, L] (host-pre-transposed bf16) -> qT/kT [512, L] -> S^T [keys, q]
  -> P^T = exp(S^T) (bf16) -> attn^T = (v|ones)^T @ P^T (Z in row 64)
  -> AllGather attn^T pair-wise -> out^T = Wo^T @ attn_full^T.

The emission order software-pipelines the attention at matmul
granularity: within a head, S(j) matmuls interleave with AV(j-3) so the
scalar-engine exp of tile j streams behind the S matmuls while the PE
keeps busy, and next-chunk projection / prev-chunk output-projection
matmul groups are spliced between heads so the PE never starves on the
exp chain.  Causal trimming skips the masked-out query columns of
diagonal key tiles in the S, exp and AV stages.
"""

import sys, types

sys.path.insert(0, "/opt/trn_rl_repo")

# antenv.axon_hooks is missing in this image; inject it so trace=True can
# reach the NTFF profiling hook (used by test.py, off by default).
if "antenv.axon_hooks" not in sys.modules:
    _hook_mod = types.ModuleType("antenv.axon_hooks")
    _hook_mod._hook = None
    def _set_hook(h):
        _hook_mod._hook = h
    def _get_hook():
        return _hook_mod._hook
    _hook_mod.set_axon_ntff_profile_hook = _set_hook
    _hook_mod.get_axon_ntff_profile_hook = _get_hook
    sys.modules["antenv.axon_hooks"] = _hook_mod
    try:
        import antenv
        antenv.axon_hooks = _hook_mod
        from trn_agent_boot.trn_boot import _ntff_profile_via_ctypes
        _set_hook(_ntff_profile_via_ctypes("/opt/axon/libaxon_pjrt.so"))
    except Exception:
        pass

import numpy as np
import ml_dtypes
import concourse.bass as bass
import concourse.mybir as mybir
import concourse.tile as tile
from concourse import bacc
from concourse.bass_utils import run_bass_kernel_spmd

B, L, D, H = 4, 2048, 1024, 16
DH = 64
N_CORES = 8
NH = 8          # heads per core
HC = NH * DH    # 512 projection cols per core
QC = 512        # q-chunk
KT = 128        # k-tile
P = 128
NQC = L // QC   # 4
NKT = L // KT   # 16
NDS = D // P    # 8 contraction tiles for projections
NO = (D // 2) // P  # 4 output-column tiles per core

F32 = mybir.dt.float32
BF16 = mybir.dt.bfloat16

PAIRS = [[0, 1], [2, 3], [4, 5], [6, 7]]

TRACE = False
LAST_EXEC_NS = None
_NC = None


def build_nc():
    nc = bacc.Bacc()

    xT = nc.declare_dram_parameter("xT", [D, L], BF16, isOutput=False)
    wq = nc.declare_dram_parameter("wq", [D, HC], BF16, isOutput=False)
    wk = nc.declare_dram_parameter("wk", [D, HC], BF16, isOutput=False)
    wv = nc.declare_dram_parameter("wv", [D, HC], BF16, isOutput=False)
    wo = nc.declare_dram_parameter("wo", [D, D // 2], BF16, isOutput=False)
    bqb = nc.declare_dram_parameter("bqb", [P, 4 * QC], F32, isOutput=False)
    bkb = nc.declare_dram_parameter("bkb", [P, 4 * QC], F32, isOutput=False)
    bvb = nc.declare_dram_parameter("bvb", [P, HC], F32, isOutput=False)
    bo = nc.declare_dram_parameter("bo", [P, NO], F32, isOutput=False)
    m01 = nc.declare_dram_parameter("m01", [P, 4 * QC], BF16, isOutput=False)
    outTh = nc.declare_dram_parameter("outTh", [D // 2, L], F32, isOutput=True)

    scale = 1.0 / np.sqrt(np.float32(DH))

    from collections import deque
    from contextlib import ExitStack
    with nc.allow_low_precision(reason="bf16 matmuls throughout by design"), \
         tile.TileContext(nc) as tc, ExitStack() as ctx:
        consts = ctx.enter_context(tc.tile_pool(name="consts", bufs=1))
        wpool = ctx.enter_context(tc.tile_pool(name="wpool", bufs=1))
        kvres = ctx.enter_context(tc.tile_pool(name="kvres", bufs=1))
        xtp = ctx.enter_context(tc.tile_pool(name="xtp", bufs=16))
        qtp = ctx.enter_context(tc.tile_pool(name="qtp", bufs=8))
        ptp = ctx.enter_context(tc.tile_pool(name="ptp", bufs=36))
        anp = ctx.enter_context(tc.tile_pool(name="anp", bufs=4))
        agsp = ctx.enter_context(tc.tile_pool(name="agsp", bufs=16))
        otp = ctx.enter_context(tc.tile_pool(name="otp", bufs=3))
        zrp = ctx.enter_context(tc.tile_pool(name="zrp", bufs=2))
        bzsb = ctx.enter_context(tc.tile_pool(name="bzsb", bufs=2))
        zdp = ctx.enter_context(tc.tile_pool(name="zdp", bufs=4, space="DRAM"))
        aginp = ctx.enter_context(tc.tile_pool(name="aginp", bufs=2, space="DRAM"))
        agoutp = ctx.enter_context(tc.tile_pool(name="agoutp", bufs=2, space="DRAM"))
        scratch = ctx.enter_context(tc.tile_pool(name="scratch", bufs=2, space="PSUM"))
        stp = ctx.enter_context(tc.tile_pool(name="stp", bufs=3, space="PSUM"))
        accp = ctx.enter_context(tc.tile_pool(name="accp", bufs=2, space="PSUM"))

        # ---- per-chunk state ----
        xT_t = {}
        qT_t = {}
        agsb_t = {}

        def emit_xT(c):
            for ds in range(NDS):
                xt = xtp.tile([P, QC], BF16, tag="xT")
                nc.sync.dma_start(
                    out=xt, in_=xT[ds * P:(ds + 1) * P, c * QC:(c + 1) * QC])
                xT_t[(ds, c)] = xt

        # ---- first-chunk input first, then constants and weights ----
        emit_xT(0)
        wq_sb = [wpool.tile([P, HC], BF16, tag=f"wq{ds}", name=f"wq{ds}") for ds in range(NDS)]
        for ds in range(NDS):
            nc.sync.dma_start(out=wq_sb[ds], in_=wq[ds * P:(ds + 1) * P, :])
        bqb_sb = consts.tile([P, 4, QC], F32, tag="bqb")
        nc.sync.dma_start(out=bqb_sb, in_=bqb[:, :].rearrange("p (t q) -> p t q", t=4))
        wk_sb = [wpool.tile([P, HC], BF16, tag=f"wk{ds}", name=f"wk{ds}") for ds in range(NDS)]
        for ds in range(NDS):
            nc.sync.dma_start(out=wk_sb[ds], in_=wk[ds * P:(ds + 1) * P, :])
        bkb_sb = consts.tile([P, 4, QC], F32, tag="bkb")
        nc.sync.dma_start(out=bkb_sb, in_=bkb[:, :].rearrange("p (t q) -> p t q", t=4))
        wv_sb = [wpool.tile([P, HC], BF16, tag=f"wv{ds}", name=f"wv{ds}") for ds in range(NDS)]
        for ds in range(NDS):
            nc.sync.dma_start(out=wv_sb[ds], in_=wv[ds * P:(ds + 1) * P, :])
        bvb_sb = consts.tile([P, NH, DH], F32, tag="bvb")
        nc.sync.dma_start(out=bvb_sb, in_=bvb[:, :].rearrange("p (h d) -> p h d", h=NH))
        m01_sb = consts.tile([P, 4, QC], BF16, tag="m01")
        nc.sync.dma_start(out=m01_sb, in_=m01[:, :].rearrange("p (m q) -> p m q", m=4))
        wo_sb = [wpool.tile([P, D // 2], BF16, tag=f"wo{k}", name=f"wo{k}") for k in range(NDS)]
        for k in range(NDS):
            nc.sync.dma_start(out=wo_sb[k], in_=wo[k * P:(k + 1) * P, :])
        bo_sb = consts.tile([P, NO], F32, tag="bo")
        nc.sync.dma_start(out=bo_sb, in_=bo[:, :])

        # ---- resident kT and v ----
        kT_sb = [kvres.tile([P, L], BF16, tag=f"kT{t}", name=f"kT{t}") for t in range(HC // P)]
        # v: per key-tile [128, NH, 65] bf16; cols 0..63 = v, col 64 = ones
        # (the ones column makes the AV matmul emit softmax Z in row 64)
        v_sb = [kvres.tile([P, NH, 65], BF16, tag=f"v{kt}", name=f"v{kt}") for kt in range(NKT)]
        for kt in range(NKT):
            nc.vector.memset(v_sb[kt][:, :, DH:DH + 1], 1.0)

        # ---- filler generators: one matmul per yield ----
        def gen_q(t, c):
            pq = scratch.tile([P, QC], F32, tag="pacc")
            for ds in range(NDS):
                nc.tensor.matmul(
                    pq, wq_sb[ds][:, t * P:(t + 1) * P], xT_t[(ds, c)],
                    start=(ds == 0), stop=(ds == NDS - 1))
                yield
            qt = qtp.tile([P, QC], BF16, tag="qT")
            nc.vector.tensor_add(qt, pq, bqb_sb[:, t, :])
            qT_t[(t, c)] = qt

        def gen_k(t, c):
            pk = scratch.tile([P, QC], F32, tag="pacc")
            for ds in range(NDS):
                nc.tensor.matmul(
                    pk, wk_sb[ds][:, t * P:(t + 1) * P], xT_t[(ds, c)],
                    start=(ds == 0), stop=(ds == NDS - 1))
                yield
            nc.vector.tensor_add(
                kT_sb[t][:, c * QC:(c + 1) * QC], pk, bkb_sb[:, t, :])

        def gen_v(sub, c):
            kt = c * (QC // P) + sub
            pv = scratch.tile([P, HC], F32, tag="pacc")
            for ds in range(NDS):
                nc.tensor.matmul(
                    pv, xT_t[(ds, c)][:, sub * P:(sub + 1) * P], wv_sb[ds],
                    start=(ds == 0), stop=(ds == NDS - 1))
                yield
            nc.vector.tensor_add(
                v_sb[kt][:, :, 0:DH],
                pv[:].rearrange("p (h d) -> p h d", h=NH),
                bvb_sb)

        def gen_oproj(o, c):
            po = scratch.tile([P, QC], F32, tag="pacc")
            for k in range(NDS):
                nc.tensor.matmul(
                    po, wo_sb[k][:, o * P:(o + 1) * P], agsb_t[(k, c)],
                    start=(k == 0), stop=(k == NDS - 1))
                yield
            ot = otp.tile([P, QC], F32, tag="ot")
            nc.scalar.activation(
                out=ot, in_=po,
                func=mybir.ActivationFunctionType.Identity,
                bias=bo_sb[:, o:o + 1], scale=1.0)
            nc.sync.dma_start(
                out=outTh[o * P:(o + 1) * P, c * QC:(c + 1) * QC], in_=ot)

        def proj_gens(c):
            for t in range(HC // P):
                yield gen_q(t, c)
                yield gen_k(t, c)
            for sub in range(QC // P):
                yield gen_v(sub, c)

        def make_filler(gens):
            q = deque(gens)
            def pop():
                while q:
                    try:
                        next(q[0])
                        return True
                    except StopIteration:
                        q.popleft()
                return False
            return pop

        # ---- attention for one head, S/AV interleaved at tile grain ----
        def attn_head(h, c, ag_in_t, fill_pop, n_pops):
            t, par = h // 2, h % 2
            njt = 4 * c + 4
            pts = []
            acc_cell = [None]

            def S_micro(j):
                m = j - 4 * c
                lo = P * m if m > 0 else 0
                st = stp.tile([P, QC], F32, tag="st")
                nc.tensor.matmul(
                    st[:, lo:QC],
                    kT_sb[t][par * DH:(par + 1) * DH, j * KT:(j + 1) * KT],
                    qT_t[(t, c)][par * DH:(par + 1) * DH, lo:QC],
                    start=True, stop=True)
                pt = ptp.tile([P, QC], BF16, tag="pt")
                nc.scalar.activation(
                    out=pt[:, lo:QC], in_=st[:, lo:QC],
                    func=mybir.ActivationFunctionType.Exp,
                    scale=float(scale))
                if m >= 0:
                    nc.vector.tensor_mul(
                        pt[:, lo:QC], pt[:, lo:QC], m01_sb[:, m, lo:QC])
                pts.append((pt, lo))

            def AV_micro(j):
                pt, lo = pts[j]
                if j == 0:
                    acc_cell[0] = accp.tile([P, QC], F32, tag="acc", name="acc")
                nc.tensor.matmul(
                    acc_cell[0][0:DH + 1, lo:QC],
                    v_sb[j][:, h, :],
                    pt[:, lo:QC],
                    start=(j == 0), stop=(j == njt - 1))

            for j in range(njt):
                S_micro(j)
                if j >= 3:
                    AV_micro(j - 3)
            for _ in range(n_pops):
                fill_pop()
            for j in range(max(njt - 3, 0), njt):
                AV_micro(j)

            # normalization: zrec = 1/Z (row 64), broadcast via DRAM DMA
            acc = acc_cell[0]
            zrec = zrp.tile([P, QC], F32, tag="zrec")
            nc.vector.reciprocal(out=zrec[DH:DH + 1, :], in_=acc[DH:DH + 1, :])
            zrow = zdp.tile([1, QC], F32, tag="zd", name="zrow")
            nc.sync.dma_start(out=zrow, in_=zrec[DH:DH + 1, :])
            bzs = bzsb.tile([DH, QC], F32, tag="bzs")
            nc.sync.dma_start(out=bzs[0:DH, :].unsqueeze(1),
                              in_=zrow.partition_broadcast(DH))
            an = anp.tile([DH, QC], BF16, tag="an")
            nc.vector.tensor_mul(an, acc[0:DH, :], bzs)
            row = t * P + par * DH
            nc.sync.dma_start(out=ag_in_t[row:row + DH, :], in_=an)

        def ag_emit(c, ag_in_t):
            ag_out_t = agoutp.tile([D, QC], BF16, tag="agout", name="agout")
            nc.gpsimd.collective_compute(
                "AllGather", mybir.AluOpType.bypass,
                replica_groups=PAIRS,
                ins=[ag_in_t.opt()], outs=[ag_out_t.opt()],
            )
            for k in range(NDS):
                ag = agsp.tile([P, QC], BF16, tag="agsb")
                nc.sync.dma_start(out=ag, in_=ag_out_t[k * P:(k + 1) * P, :])
                agsb_t[(k, c)] = ag

        # ---- chunk 0 projections up-front ----
        p0 = make_filler(proj_gens(0))
        while p0():
            pass

        # ---- main pipeline over chunks ----
        for c in range(NQC):
            if c + 1 < NQC:
                emit_xT(c + 1)
            gens = list(proj_gens(c + 1)) if c + 1 < NQC else []
            if c > 0:
                gens += [gen_oproj(o, c - 1) for o in range(NO)]
            n_micros = 8 * len(gens)
            fill_pop = make_filler(gens)
            n_pops = (n_micros + NH - 1) // NH if n_micros else 0
            ag_in_t = aginp.tile([HC, QC], BF16, tag="agin", name="agin")
            for h in range(NH):
                attn_head(h, c, ag_in_t, fill_pop, n_pops)
            while fill_pop():
                pass
            ag_emit(c, ag_in_t)

        # ---- last chunk's output projection ----
        last = make_filler([gen_oproj(o, NQC - 1) for o in range(NO)])
        while last():
            pass

    nc.compile()
    return nc


def _make_in_maps(x, Wq, bq, Wk, bk, Wv, bv, Wo, bo, mask):
    ref = np.tril(np.ones((L, L), dtype=np.int32))[None, None]
    assert np.array_equal(np.asarray(mask), ref), "mask must be causal"

    # m01 patterns for the 4 diagonal k-tiles of a q-chunk:
    # pattern_m[p, f] = 1 if p <= f - 128*m
    pf = np.arange(QC)[None, :] - np.arange(P)[:, None]
    m01 = np.concatenate(
        [(pf >= P * m).astype(np.float32) for m in range(4)], axis=1)

    def bcast_bias(b_slice):
        # [512] -> [128 partitions, 4 t-tiles, 512 cols] broadcast over cols
        return np.ascontiguousarray(
            np.broadcast_to(
                np.asarray(b_slice).reshape(4, P, 1).transpose(1, 0, 2),
                (P, 4, QC)).reshape(P, 4 * QC)).astype(np.float32)

    in_maps = []
    for c in range(N_CORES):
        b, g = c // 2, c % 2
        cols = slice(HC * g, HC * g + HC)
        half = slice((D // 2) * g, (D // 2) * g + D // 2)
        in_maps.append({
            "xT": np.ascontiguousarray(np.asarray(x[b]).T).astype(ml_dtypes.bfloat16),
            "wq": np.ascontiguousarray(np.asarray(Wq)[:, cols]).astype(ml_dtypes.bfloat16),
            "wk": np.ascontiguousarray(np.asarray(Wk)[:, cols]).astype(ml_dtypes.bfloat16),
            "wv": np.ascontiguousarray(np.asarray(Wv)[:, cols]).astype(ml_dtypes.bfloat16),
            "wo": np.ascontiguousarray(np.asarray(Wo)[:, half]).astype(ml_dtypes.bfloat16),
            "bqb": bcast_bias(np.asarray(bq)[cols]),
            "bkb": bcast_bias(np.asarray(bk)[cols]),
            "bvb": np.ascontiguousarray(
                np.broadcast_to(np.asarray(bv)[cols], (P, HC))).astype(np.float32),
            "bo": np.ascontiguousarray(
                np.asarray(bo)[half].reshape(NO, P).T.astype(np.float32)),
            "m01": m01.astype(ml_dtypes.bfloat16),
        })
    return in_maps


def kernel(x, Wq, bq, Wk, bk, Wv, bv, Wo, bo, mask):
    global _NC, LAST_EXEC_NS
    if _NC is None:
        _NC = build_nc()
    in_maps = _make_in_maps(x, Wq, bq, Wk, bk, Wv, bv, Wo, bo, mask)
    r = run_bass_kernel_spmd(
        _NC, in_maps, core_ids=list(range(N_CORES)), trace=TRACE)
    LAST_EXEC_NS = r.exec_time_ns
    out = np.empty((B, L, D), dtype=np.float32)
    for b in range(B):
        outT = np.concatenate(
            [r.results[2 * b]["outTh"], r.results[2 * b + 1]["outTh"]], axis=0)
        out[b] = outT.T
    return out
